# revision 1
# baseline (speedup 1.0000x reference)
"""Fused decoder block (LN->QKV->cache-merge attention->proj->LN->MLP) on 8
Trainium2 NeuronCores, data-parallel over the batch (2 rows/core).

Key ideas:
- softmax is permutation-invariant over keys, so instead of scattering new
  k/v into the cache at masked slots, attend over [cache keys (masked slots
  suppressed) ++ all new keys]. The suppression is a -1e4 additive bias
  folded into the exp() activation's per-partition bias operand - zero cost.
- everything runs feature-major ([C, T] activations) so no transposes are
  needed anywhere: W as lhsT keeps activations feature-major, activations as
  lhsT produce token-major (used only for v).
- scores are computed keys-major [keys, queries]; exp'd probabilities feed
  P@V directly as the moving operand with natural-layout V as weights; an
  extra ones column on V accumulates the softmax denominator in the same
  matmuls. Per-query normalization is broadcast across partitions with a
  rank-1 PE outer product.
- LN stats (sum, sum-sq) via ones-column matmuls in fp32r; LN scale/shift
  applied via two rank<=2 PE broadcasts (g (x) rstd, g (x) -mu*rstd + b (x) 1).
- matmul dtypes: fp32r (full PE rate, ~13-bit mantissa) everywhere except
  attention qk/PV and fc2 which run bf16.
"""

import numpy as np
import ml_dtypes

B, NP, N, C, H = 16, 512, 1024, 1024, 16
HD = C // H            # 64
HID = 4 * C            # 4096
EPS = 1e-5
NCORES = 8
RPC = B // NCORES      # batch rows per core
T = NP                 # queries per row
CT = C // 128          # feature tiles
KTC = N // 128         # cache key tiles
KTN = T // 128         # new key tiles
KTA = KTC + KTN        # all key tiles
HPAIR = H // 2         # head pairs
SCALE = HD ** -0.5
MASKB = -10000.0

_state = {}


def _build_module():
    import concourse.tile as tile
    from concourse import bacc, mybir

    f32 = mybir.dt.float32
    f32r = mybir.dt.float32r
    bf16 = mybir.dt.bfloat16
    AF = mybir.ActivationFunctionType
    OP = mybir.AluOpType

    nc = bacc.Bacc("TRN2", target_bir_lowering=False, debug=False)

    xT = nc.dram_tensor("xT", [RPC, C, T], f32r, kind="ExternalInput")
    kTc = nc.dram_tensor("kTc", [RPC, H, HD, N], bf16, kind="ExternalInput")
    vc = nc.dram_tensor("vc", [RPC, H, N, HD + 1], bf16, kind="ExternalInput")
    mb = nc.dram_tensor("mb", [RPC, N], f32, kind="ExternalInput")
    wqkv = nc.dram_tensor("wqkv", [C, 3 * C], f32r, kind="ExternalInput")
    wproj = nc.dram_tensor("wproj", [C, C], f32r, kind="ExternalInput")
    wfc1 = nc.dram_tensor("wfc1", [C, HID], f32r, kind="ExternalInput")
    wfc2 = nc.dram_tensor("wfc2", [HID, C], bf16, kind="ExternalInput")
    bqkv = nc.dram_tensor("bqkv", [3 * C], f32, kind="ExternalInput")
    bproj = nc.dram_tensor("bproj", [C], f32, kind="ExternalInput")
    bfc1 = nc.dram_tensor("bfc1", [HID], f32, kind="ExternalInput")
    bfc2 = nc.dram_tensor("bfc2", [C], f32, kind="ExternalInput")
    # LN gains/biases, reshaped [CT, 128] host-side
    n1g = nc.dram_tensor("n1g", [CT, 128], f32r, kind="ExternalInput")
    n1b = nc.dram_tensor("n1b", [CT, 128], f32r, kind="ExternalInput")
    n2g = nc.dram_tensor("n2g", [CT, 128], f32r, kind="ExternalInput")
    n2b = nc.dram_tensor("n2b", [CT, 128], f32r, kind="ExternalInput")
    ones = nc.dram_tensor("ones", [128, 512], f32r, kind="ExternalInput")
    outT = nc.dram_tensor("outT", [RPC, C, T], f32, kind="ExternalOutput")

    from contextlib import ExitStack
    with nc.allow_low_precision(reason="deliberate bf16/f32r staging; accumulation stays fp32 in PSUM"), \
         tile.TileContext(nc, pool_alloc_mode="queue") as tc, ExitStack() as es:
        # ---------- constants resident for the whole kernel ----------
        consts = es.enter_context(tc.tile_pool(name="consts", bufs=1))
        ones_sb = consts.tile([128, 512], f32r)
        nc.gpsimd.dma_start(ones_sb[:], ones.ap())
        gb1 = consts.tile([2, CT, 128], f32r)
        nc.gpsimd.dma_start(gb1[0:1], n1g.ap()[None])
        nc.gpsimd.dma_start(gb1[1:2], n1b.ap()[None])
        gb2 = consts.tile([2, CT, 128], f32r)
        nc.gpsimd.dma_start(gb2[0:1], n2g.ap()[None])
        nc.gpsimd.dma_start(gb2[1:2], n2b.ap()[None])
        bqkv_sb = consts.tile([128, 16], f32)  # q,k bias columns per fchunk
        nc.gpsimd.dma_start(bqkv_sb[:], bqkv.ap()[0:2048].rearrange("(fc p) -> p fc", p=128))
        vb_sb = consts.tile([128, 2, 512], f32)  # v bias broadcast over tokens
        for ch in range(2):
            nc.gpsimd.dma_start(
                vb_sb[:, ch, :],
                bqkv.ap()[2048 + ch * 512: 2048 + (ch + 1) * 512][None].to_broadcast((128, 512)))
        bproj_sb = consts.tile([128, CT], f32)
        nc.gpsimd.dma_start(bproj_sb[:], bproj.ap().rearrange("(co p) -> p co", p=128))
        bfc1_sb = consts.tile([128, HID // 128], f32)
        nc.gpsimd.dma_start(bfc1_sb[:], bfc1.ap().rearrange("(ht p) -> p ht", p=128))
        bfc2_sb = consts.tile([128, CT], f32)
        nc.gpsimd.dma_start(bfc2_sb[:], bfc2.ap().rearrange("(co p) -> p co", p=128))
        eps_sb = consts.tile([1, 1], f32)
        nc.vector.memset(eps_sb[:], EPS)
        mb_sb = consts.tile([128, RPC, KTC], f32)
        for r in range(RPC):
            nc.gpsimd.dma_start(mb_sb[:, r, :], mb.ap()[r].rearrange("(kt p) -> p kt", p=128))

        # Pools opened/closed at phase boundaries; queue mode allows
        # non-LIFO release so each buffer spans exactly its lifetime.
        def open_pool(nm):
            cm = tc.tile_pool(name=nm, bufs=1)
            return cm, cm.__enter__()

        def close_pool(cm):
            cm.__exit__(None, None, None)

        dram_pool = es.enter_context(tc.tile_pool(name="x2d", bufs=1, space="DRAM"))
        x2ds = [[dram_pool.tile([128, T], f32, tag=f"x2d{r}_{c}", name=f"x2d{r}_{c}")
                 for c in range(CT)] for r in range(RPC)]
        cm_oT, p_oT = open_pool("p_oT")
        oTs = [[p_oT.tile([128, T], f32r, tag=f"oT{r}_{c}", name=f"oT{r}_{c}")
                for c in range(CT)] for r in range(RPC)]
        cm_h, p_h = open_pool("p_h")
        # prefetch first q/k weight chunks while LN1 runs
        cm_w0 = tc.tile_pool(name="p_w0", bufs=1)
        p_w0 = cm_w0.__enter__()
        w0_tiles = {}
        for fc in (0, 8):
            wt = p_w0.tile([128, CT, 128], f32r, tag=f"w0_{fc}", name=f"w0_{fc}")
            nc.sync.dma_start(
                wt[:], wqkv.ap()[:, fc * 128:(fc + 1) * 128]
                .rearrange("(ct p) f -> p ct f", p=128))
            w0_tiles[fc] = wt

        cm_xa, p_xa = open_pool("p_xa")
        xTs = [[p_xa.tile([128, T], f32r, tag=f"xT{r}_{ct}", name=f"xT{r}_{ct}")
                for ct in range(CT)] for r in range(RPC)]
        hTs = [[p_h.tile([128, T], f32r, tag=f"hT{r}_{ct}", name=f"hT{r}_{ct}")
                for ct in range(CT)] for r in range(RPC)]

        def layernorm(src_tiles, dst_tiles, gb, lnp, lnps, sbufs=2):
            """Feature-major layernorm src -> dst (lists of CT [128,T] tiles)."""
            s_ps = lnps.tile([1, T], f32, tag="s_ps", name="s_ps", bufs=sbufs)
            ss_ps = lnps.tile([1, T], f32, tag="ss_ps", name="ss_ps", bufs=sbufs)
            for ct in range(CT):
                nc.tensor.matmul(s_ps[:], ones_sb[:, 0:1], src_tiles[ct][:],
                                 start=(ct == 0), stop=(ct == CT - 1))
            sqs = []
            for ct in range(CT):
                sq = lnp.tile([128, T], f32r, tag="sq", name="sq", bufs=2)
                nc.vector.tensor_mul(sq[:], src_tiles[ct][:].bitcast(f32),
                                     src_tiles[ct][:].bitcast(f32))
                sqs.append(sq)
            for ct in range(CT):
                nc.tensor.matmul(ss_ps[:], ones_sb[:, 0:1], sqs[ct][:],
                                 start=(ct == 0), stop=(ct == CT - 1))
            st = lnp.tile([97, T], f32, tag="st", name="st", bufs=2)
            mean, msq, var, std = st[0:1, :], st[32:33, :], st[64:65, :], st[96:97, :]
            nc.scalar.mul(mean, s_ps[:], 1.0 / C)
            nc.vector.tensor_mul(msq, mean, mean)
            nc.vector.scalar_tensor_tensor(var, ss_ps[:], 1.0 / C, msq,
                                           OP.mult, OP.subtract)
            nc.scalar.activation(std, var, AF.Sqrt, bias=eps_sb[:])
            rstd = lnp.tile([1, T], f32r, tag="rstd", name="rstd", bufs=2)
            nc.vector.reciprocal(rstd[:], std)
            nmr = lnp.tile([2, T], f32r, tag="nmr", name="nmr", bufs=2)
            nc.vector.scalar_tensor_tensor(nmr[0:1, :], mean, -1.0,
                                           rstd[:].bitcast(f32), OP.mult, OP.mult)
            nc.sync.dma_start(nmr[1:2, :], ones.ap()[0:1, :])
            for ct in range(CT):
                a_ps = lnps.tile([128, T], f32, tag="a_ps", name="a_ps", bufs=2)
                nc.tensor.matmul(a_ps[:], gb[0:1, ct, :], rstd[:],
                                 start=True, stop=True)
                b_ps = lnps.tile([128, T], f32, tag="b_ps", name="b_ps", bufs=2)
                nc.tensor.matmul(b_ps[:], gb[:, ct, :], nmr[:],
                                 start=True, stop=True)
                t1 = lnp.tile([128, T], f32, tag="t1", name="t1", bufs=2)
                nc.vector.tensor_mul(t1[:], src_tiles[ct][:].bitcast(f32), a_ps[:])
                nc.vector.tensor_add(dst_tiles[ct][:], t1[:], b_ps[:])

        # ================= LN1 =================
        with tc.tile_pool(name="ln1", bufs=1) as lnp, \
             tc.tile_pool(name="ln1ps", bufs=1, space="PSUM") as lnps:
            for r in range(RPC):
                for ct in range(CT):
                    nc.sync.dma_start(xTs[r][ct][:], xT.ap()[r, ct * 128:(ct + 1) * 128, :])
                layernorm(xTs[r], hTs[r], gb1, lnp, lnps)

        # ========== QKV + Attention + Proj (one PSUM scope, overlapped) ==========
        close_pool(cm_xa)
        cm_qk, p_qk = open_pool("p_qk")
        cm_vn, p_vn = open_pool("p_vn")
        qTs = [[p_qk.tile([128, T], bf16, tag=f"qT{r}_{c}", name=f"qT{r}_{c}")
                for c in range(CT)] for r in range(RPC)]
        kTs = [[p_qk.tile([128, T], bf16, tag=f"kT{r}_{c}", name=f"kT{r}_{c}")
                for c in range(CT)] for r in range(RPC)]
        vns = [p_vn.tile([128, KTN, H, HD + 1], bf16, tag=f"vn{r}", name=f"vn{r}")
               for r in range(RPC)]
        def emit_qkv_chunk(fc, wqk_pool, mps):
            """One 128-col chunk of q or k for both rows."""
            if fc in w0_tiles:
                wt = w0_tiles[fc]
            else:
                wt = wqk_pool.tile([128, CT, 128], f32r, tag="wqk", name="wqk", bufs=4)
                nc.sync.dma_start(
                    wt[:], wqkv.ap()[:, fc * 128:(fc + 1) * 128]
                    .rearrange("(ct p) f -> p ct f", p=128))
            for r in range(RPC):
                ps = mps.tile([128, T], f32, tag="mm", name="mm", bufs=1)
                for ct in range(CT):
                    nc.tensor.matmul(ps[:], wt[:, ct, :], hTs[r][ct][:],
                                     start=(ct == 0), stop=(ct == CT - 1))
                dst = qTs[r][fc] if fc < 8 else kTs[r][fc - 8]
                nc.vector.tensor_scalar(
                    dst[:], ps[:], bqkv_sb[:, fc:fc + 1], None, OP.add)

        def emit_attention(hp, r, akv, asb, mps):
            kc = akv.tile([128, N], bf16, tag="kc", name="kc", bufs=3)
            nc.sync.dma_start(kc[0:64, :], kTc.ap()[r, 2 * hp])
            nc.sync.dma_start(kc[64:128, :], kTc.ap()[r, 2 * hp + 1])
            vcs = [akv.tile([128, KTC, HD + 1], bf16, tag="vcc", name="vcc", bufs=4)
                   for _ in range(2)]
            for hh in range(2):
                nc.sync.dma_start(
                    vcs[hh][:], vc.ap()[r, 2 * hp + hh]
                    .rearrange("(kt p) d -> p kt d", p=128))
            pv = [mps.tile([HD + 1, T], f32, tag="pv", name=f"pv{hh}", bufs=2)
                  for hh in range(2)]
            for kt in range(KTA):
                if kt < KTC:
                    lA = kc[0:64, kt * 128:(kt + 1) * 128]
                    lB = kc[64:128, kt * 128:(kt + 1) * 128]
                    bias = [mb_sb[:, r, kt:kt + 1], mb_sb[:, r, kt:kt + 1]]
                else:
                    ktn = kt - KTC
                    lA = kTs[r][hp][0:64, ktn * 128:(ktn + 1) * 128]
                    lB = kTs[r][hp][64:128, ktn * 128:(ktn + 1) * 128]
                    bias = [0.0, 0.0]
                s_A = mps.tile([128, T], f32, tag="sA", name="sA", bufs=2)
                s_B = mps.tile([128, T], f32, tag="sB", name="sB", bufs=2)
                nc.tensor.matmul(s_A[:], lA, qTs[r][hp][0:64, :],
                                 start=True, stop=True, tile_position=(0, 0))
                nc.tensor.matmul(s_B[:], lB, qTs[r][hp][64:128, :],
                                 start=True, stop=True, tile_position=(64, 0))
                for hh, s_ps_t in ((0, s_A), (1, s_B)):
                    p_t = asb.tile([128, T], bf16, tag="p", name="p", bufs=8)
                    nc.scalar.activation(p_t[:], s_ps_t[:], AF.Exp,
                                         bias=bias[hh], scale=SCALE)
                    lv = vcs[hh][:, kt, :] if kt < KTC else \
                        vns[r][:, kt - KTC, 2 * hp + hh, :]
                    nc.tensor.matmul(pv[hh][:], lv, p_t[:],
                                     start=(kt == 0), stop=(kt == KTA - 1))
            for hh in range(2):
                rd = asb.tile([1, T], f32r, tag="rd", name="rd", bufs=2)
                nc.vector.reciprocal(rd[:], pv[hh][HD:HD + 1, :])
                bc = mps.tile([HD, T], f32, tag="bc", name="bc", bufs=1)
                nc.tensor.matmul(bc[:], ones_sb[0:1, 0:HD], rd[:],
                                 start=True, stop=True)
                bc_sb = asb.tile([HD, T], f32, tag="bcs", name="bcs", bufs=2)
                nc.vector.tensor_copy(bc_sb[:], bc[:])
                half = oTs[r][hp][64 * hh:64 * (hh + 1), :]
                nc.vector.tensor_mul(half, pv[hh][0:HD, :], bc_sb[:])

        with tc.tile_pool(name="wqk", bufs=1) as wqk_pool, \
             tc.tile_pool(name="attn_kv", bufs=1) as akv, \
             tc.tile_pool(name="attn_sb", bufs=1) as asb, \
             tc.tile_pool(name="xb", bufs=1) as xb_pool, \
             tc.tile_pool(name="merged_ps", bufs=1, space="PSUM") as mps:
            emit_qkv_chunk(0, wqk_pool, mps)
            emit_qkv_chunk(8, wqk_pool, mps)
            for r in range(RPC):
                nc.vector.memset(vns[r][:, :, :, HD:HD + 1], 1.0)
            for ch in range(4):
                wv = wqk_pool.tile([128, CT, 256], f32r, tag="wv", name="wv", bufs=2)
                nc.sync.dma_start(
                    wv[:], wqkv.ap()[:, 2048 + ch * 256: 2048 + (ch + 1) * 256]
                    .rearrange("(ct p) f -> p ct f", p=128))
                for r in range(RPC):
                    for tt in range(KTN):
                        ps = mps.tile([128, 256], f32, tag="mm", name="mmv", bufs=1)
                        for ct in range(CT):
                            nc.tensor.matmul(
                                ps[:], hTs[r][ct][:, tt * 128:(tt + 1) * 128],
                                wv[:, ct, :], start=(ct == 0), stop=(ct == CT - 1))
                        nc.vector.tensor_add(
                            vns[r][:, tt, ch * 4:(ch + 1) * 4, 0:HD],
                            ps[:].rearrange("p (h d) -> p h d", h=4),
                            vb_sb[:, ch // 2, ch % 2 * 256:(ch % 2 + 1) * 256]
                            .rearrange("p (h d) -> p h d", h=4))
            # per head pair: q chunk, k chunk, then attention for both rows
            for hp in range(HPAIR):
                if hp > 0:
                    emit_qkv_chunk(hp, wqk_pool, mps)
                    emit_qkv_chunk(8 + hp, wqk_pool, mps)
                for r in range(RPC):
                    emit_attention(hp, r, akv, asb, mps)
        close_pool(cm_vn)
        close_pool(cm_qk)
        cm_w0.__exit__(None, None, None)
        close_pool(cm_h)

        # ================= Proj + residual =================
        cm_h2, p_h2 = open_pool("p_h2")
        h2Ts = [[p_h2.tile([128, T], f32r, tag=f"h2T{r}_{ct}", name=f"h2T{r}_{ct}")
                 for ct in range(CT)] for r in range(RPC)]
        cm_w1 = tc.tile_pool(name="w1", bufs=1)
        w1_pool = cm_w1.__enter__()
        NHT_EARLY = 4
        cm_ge, p_ge = open_pool("p_ge")
        gearly = [p_ge.tile([128, NHT_EARLY, T], bf16, tag=f"ge{r}", name=f"ge{r}")
                  for r in range(RPC)]
        cm_x2, p_x2 = open_pool("p_x2")
        x2Ts = [[p_x2.tile([128, T], f32r, tag=f"x2T{r}_{ct}", name=f"x2T{r}_{ct}")
                 for ct in range(CT)] for r in range(RPC)]
        with tc.tile_pool(name="wp", bufs=1) as wp_pool, \
             tc.tile_pool(name="xb2p", bufs=1) as xbp_pool, \
             tc.tile_pool(name="ln2", bufs=1) as lnp2, \
             tc.tile_pool(name="proj_ps", bufs=1, space="PSUM") as pps:
            for r in range(RPC):
                for co in range(CT):
                    wt = wp_pool.tile([128, CT, 128], f32r, tag="wp", name="wp", bufs=3)
                    nc.sync.dma_start(
                        wt[:], wproj.ap()[:, co * 128:(co + 1) * 128]
                        .rearrange("(ci p) f -> p ci f", p=128))
                    ps = pps.tile([128, T], f32, tag="proj", name="proj", bufs=2)
                    for ci in range(CT):
                        nc.tensor.matmul(ps[:], wt[:, ci, :], oTs[r][ci][:],
                                         start=(ci == 0), stop=(ci == CT - 1))
                    xb = xbp_pool.tile([128, T], f32, tag="xb", name="xb", bufs=4)
                    nc.gpsimd.dma_start(
                        xb[:], xT.ap()[r, co * 128:(co + 1) * 128, :].bitcast(f32))
                    nc.vector.scalar_tensor_tensor(
                        x2Ts[r][co][:], ps[:], bproj_sb[:, co:co + 1],
                        xb[:], OP.add, OP.add)
                    nc.gpsimd.dma_start(x2ds[r][co][:], x2Ts[r][co][:].bitcast(f32))
                layernorm(x2Ts[r], h2Ts[r], gb2, lnp2, pps, sbufs=1)
            # first FC1 chunks share this psum scope to overlap the LN2 tail
            for ht in range(NHT_EARLY):
                wt = w1_pool.tile([128, CT, 128], f32r, tag="w1", name="w1", bufs=4)
                nc.sync.dma_start(
                    wt[:], wfc1.ap()[:, ht * 128:(ht + 1) * 128]
                    .rearrange("(ct p) f -> p ct f", p=128))
                for r in range(RPC):
                    ps = pps.tile([128, T], f32, tag="proj", name="fc1e", bufs=2)
                    for ct in range(CT):
                        nc.tensor.matmul(ps[:], wt[:, ct, :], h2Ts[r][ct][:],
                                         start=(ct == 0), stop=(ct == CT - 1))
                    nc.scalar.activation(gearly[r][:, ht, :], ps[:], AF.Gelu,
                                         bias=bfc1_sb[:, ht:ht + 1])
        close_pool(cm_x2)

        # ================= FC1 + gelu =================
        cm_gs = []
        ghalves = [[], []]
        for r in range(RPC):
            for half in range(2):
                cm_g, p_g = open_pool(f"p_g{r}_{half}")
                cm_gs.append(cm_g)
                ghalves[r].append(p_g.tile([128, (HID // 128 - NHT_EARLY) // 2, T],
                                           bf16, tag=f"gT{r}{half}", name=f"gT{r}{half}"))

        def gslice(r, ht):
            if ht < NHT_EARLY:
                return gearly[r][:, ht, :]
            h = ht - NHT_EARLY
            nh = (HID // 128 - NHT_EARLY) // 2
            return ghalves[r][h // nh][:, h % nh, :]
        NHT = HID // 128
        with tc.tile_pool(name="w2", bufs=1) as w2_pool, \
             tc.tile_pool(name="xb2", bufs=1) as xb2_pool, \
             tc.tile_pool(name="osb", bufs=1) as osb, \
             tc.tile_pool(name="fc1_ps", bufs=1, space="PSUM") as f1ps:
            f2ps = f1ps
            for ht in range(NHT_EARLY, HID // 128):
                wt = w1_pool.tile([128, CT, 128], f32r, tag="w1", name="w1", bufs=4)
                nc.sync.dma_start(
                    wt[:], wfc1.ap()[:, ht * 128:(ht + 1) * 128]
                    .rearrange("(ct p) f -> p ct f", p=128))
                for r in range(RPC):
                    ps = f1ps.tile([128, T], f32, tag="fc1", name="fc1", bufs=4)
                    for ct in range(CT):
                        nc.tensor.matmul(ps[:], wt[:, ct, :], h2Ts[r][ct][:],
                                         start=(ct == 0), stop=(ct == CT - 1))
                    nc.scalar.activation(gslice(r, ht), ps[:], AF.Gelu,
                                         bias=bfc1_sb[:, ht:ht + 1])

            # ---- FC2 + residual -> out (same psum scope) ----
            for co in range(CT):
                whs = []
                for half in range(2):
                    wh = w2_pool.tile([128, NHT // 2, 128], bf16, tag="w2",
                                      name="w2", bufs=3)
                    nc.sync.dma_start(
                        wh[:], wfc2.ap()[half * 2048:(half + 1) * 2048,
                                         co * 128:(co + 1) * 128]
                        .rearrange("(ht p) f -> p ht f", p=128))
                    whs.append(wh)
                for r in range(RPC):
                    ps = f2ps.tile([128, T], f32, tag="fc2", name="fc2", bufs=3)
                    for ht in range(NHT):
                        nc.tensor.matmul(ps[:], whs[ht // (NHT // 2)][:, ht % (NHT // 2), :],
                                         gslice(r, ht),
                                         start=(ht == 0), stop=(ht == NHT - 1))
                    xb2 = xb2_pool.tile([128, T], f32, tag="xb2", name="xb2", bufs=3)
                    nc.gpsimd.dma_start(xb2[:], x2ds[r][co][:])
                    ot = osb.tile([128, T], f32, tag="ot", name="ot", bufs=2)
                    nc.vector.scalar_tensor_tensor(
                        ot[:], ps[:], bfc2_sb[:, co:co + 1], xb2[:], OP.add, OP.add)
                    nc.sync.dma_start(outT.ap()[r, co * 128:(co + 1) * 128, :], ot[:])
        for cm_g in reversed(cm_gs):
            close_pool(cm_g)
        close_pool(cm_ge)
        cm_w1.__exit__(None, None, None)
        close_pool(cm_h2)
        close_pool(cm_oT)

    nc.compile()
    return nc


class _Runner:
    """Hold the compiled PJRT executable (mirrors bass2jax.run_bass_via_pjrt)."""

    def __init__(self, nc, n_cores):
        import jax
        from jax.sharding import Mesh, PartitionSpec
        from jax.experimental.shard_map import shard_map
        import concourse.mybir as mybir
        from concourse.bass2jax import (
            install_neuronx_cc_hook, partition_id_tensor, _bass_exec_p)

        install_neuronx_cc_hook()
        self.jax = jax
        self.n_cores = n_cores
        partition_name = nc.partition_id_tensor.name if nc.partition_id_tensor else None
        in_names, out_names, out_avals, zero_outs = [], [], [], []
        for alloc in nc.m.functions[0].allocations:
            if not isinstance(alloc, mybir.MemoryLocationSet):
                continue
            name = alloc.memorylocations[0].name
            if alloc.kind == "ExternalInput":
                if name != partition_name:
                    in_names.append(name)
            elif alloc.kind == "ExternalOutput":
                shape = tuple(alloc.tensor_shape)
                dtype = mybir.dt.np(alloc.dtype)
                out_names.append(name)
                out_avals.append(jax.core.ShapedArray(shape, dtype))
                zero_outs.append(np.zeros(shape, dtype))
        self.in_names, self.out_names = in_names, out_names
        self.out_avals, self.zero_outs = out_avals, zero_outs
        self.n_params = len(in_names)
        all_names = in_names + out_names
        if partition_name is not None:
            all_names.append(partition_name)

        def _body(*args):
            operands = list(args)
            if partition_name is not None:
                operands.append(partition_id_tensor())
            return tuple(
                _bass_exec_p.bind(
                    *operands,
                    out_avals=tuple(out_avals),
                    in_names=tuple(all_names),
                    out_names=tuple(out_names),
                    lowering_input_output_aliases=(),
                    sim_require_finite=True,
                    sim_require_nnan=True,
                    nc=nc,
                ))

        devices = jax.devices()[:n_cores]
        assert len(devices) == n_cores, f"need {n_cores} cores, have {len(jax.devices())}"
        mesh = Mesh(np.asarray(devices), ("core",))
        n_outs = len(out_names)
        self._fn = jax.jit(
            shard_map(_body, mesh=mesh,
                      in_specs=(PartitionSpec("core"),) * (self.n_params + n_outs),
                      out_specs=(PartitionSpec("core"),) * n_outs,
                      check_rep=False),
            keep_unused=True)

    def prepare(self, in_maps):
        np_ = np
        per_core = [[np_.asarray(m[n]) for n in self.in_names] for m in in_maps]
        concat_in = [
            np_.concatenate([per_core[c][i] for c in range(self.n_cores)], axis=0)
            for i in range(self.n_params)]
        concat_zeros = [
            np_.zeros((self.n_cores * z.shape[0], *z.shape[1:]), z.dtype)
            for z in self.zero_outs]
        return self.jax.device_put(concat_in + concat_zeros)

    def run(self, prepared):
        out = self._fn(*prepared)
        self.jax.block_until_ready(out)
        return out

    def results(self, out_arrs):
        return [
            {name: np.asarray(out_arrs[i]).reshape(
                self.n_cores, *self.out_avals[i].shape)[c]
             for i, name in enumerate(self.out_names)}
            for c in range(self.n_cores)]


def _get_runner():
    if "runner" not in _state:
        nc = _build_module()
        _state["nc"] = nc
        _state["runner"] = _Runner(nc, NCORES)
    return _state["runner"]


def _prepare_in_maps(x, cache_k, cache_v, update_mask, qkv_w, qkv_b, proj_w,
                     proj_b, n1_g, n1_b, n2_g, n2_b, fc1_w, fc1_b, fc2_w, fc2_b):
    bf = ml_dtypes.bfloat16
    f32 = np.float32
    xT = np.ascontiguousarray(np.swapaxes(np.asarray(x, f32), 1, 2))          # [B,C,T]
    kTc = np.ascontiguousarray(np.swapaxes(np.asarray(cache_k, f32), 2, 3)).astype(bf)
    vc_f = np.asarray(cache_v, f32)
    vc = np.concatenate([vc_f, np.ones((*vc_f.shape[:3], 1), f32)], axis=3).astype(bf)
    mbias = np.where(np.asarray(update_mask, bool), MASKB, 0.0).astype(f32)
    shared = dict(
        wqkv=np.asarray(qkv_w, f32), wproj=np.asarray(proj_w, f32),
        wfc1=np.asarray(fc1_w, f32), wfc2=np.asarray(fc2_w, f32).astype(bf),
        bqkv=np.asarray(qkv_b, f32), bproj=np.asarray(proj_b, f32),
        bfc1=np.asarray(fc1_b, f32), bfc2=np.asarray(fc2_b, f32),
        n1g=np.asarray(n1_g, f32).reshape(CT, 128),
        n1b=np.asarray(n1_b, f32).reshape(CT, 128),
        n2g=np.asarray(n2_g, f32).reshape(CT, 128),
        n2b=np.asarray(n2_b, f32).reshape(CT, 128),
        ones=np.ones((128, 512), f32),
    )
    in_maps = []
    for c in range(NCORES):
        s = slice(c * RPC, (c + 1) * RPC)
        in_maps.append(dict(shared, xT=xT[s], kTc=kTc[s], vc=vc[s], mb=mbias[s]))
    return in_maps


def kernel(**inputs) -> np.ndarray:
    runner = _get_runner()
    in_maps = _prepare_in_maps(**inputs)
    prepared = runner.prepare(in_maps)
    out = runner.run(prepared)
    res = runner.results(out)
    full = np.empty((B, NP, C), np.float32)
    for c in range(NCORES):
        for r in range(RPC):
            full[c * RPC + r] = res[c]["outT"][r].T
    return full



# revision 15
# speedup vs baseline: 1.4068x; 1.4068x over previous
"""Fused decoder block (LN->QKV->cache-merge attention->proj->LN->MLP) on 8
Trainium2 NeuronCores, data-parallel over the batch (2 rows/core).

Key ideas vs the bf16/f32r baseline:
- update_mask has exactly NP active slots per row, so the kept cache keys are
  gathered HOST-side: attention runs over 512 kept + 512 new = 1024 keys
  (was 1536 with -1e4 masking), with no mask bias at all.
- All big GEMMs (QKV, proj, FC1, FC2) run fp8e4 with DoubleRow perf mode:
  [128, 2, N] operand pairs give 256-deep contraction per instruction at
  0.5 cycles/row = 4x the bf16 rate. Weights are pre-scaled x64 host-side
  (fp8e4 max is 240); descales fold into existing TSP/activation scales.
- LN2 output feeds FC1 as an fp8 hi+lo pair at a common x16 scale
  (hi = fp8(16*h2), lo = fp8(16*h2 - hi)), so one weight tensor serves both
  and the effective activation error is ~0.1%.
- attention q/k/v and probabilities stay bf16 (perm-invariant softmax, no
  mask). The attention output is scaled x32 into fp8 via the existing
  rank-1 denominator broadcast (a 32.0-column outer product).
- k-bias is dropped entirely (adds a per-query constant to scores ->
  softmax invariant); proj bias is folded host-side into the residual
  source (xTb = x + proj_b).
- cheap elementwise work (LN squares/muls, lo16 split, bc copies, v casts,
  final residual add) runs on the idle GpSimd/Pool engine.
"""

import numpy as np
import ml_dtypes

B, NP, N, C, H = 16, 512, 1024, 1024, 16
HD = C // H            # 64
HID = 4 * C            # 4096
EPS = 1e-5
NCORES = 8
RPC = B // NCORES      # batch rows per core
T = NP                 # queries per row
CT = C // 128          # feature tiles
KTC = NP // 128        # kept cache key tiles (512 kept keys)
KTN = T // 128         # new key tiles
KTA = KTC + KTN        # all key tiles (8)
HPAIR = H // 2         # head pairs
SCALE = HD ** -0.5
SW = 64.0              # fp8 weight scale
SO = 32.0              # fp8 attention-output scale
SH2 = 16.0             # fp8 h2 hi/lo scale
NHT = HID // 128       # 32 fc1 output chunks

_state = {}


def _build_module():
    import concourse.tile as tile
    from concourse import bacc, mybir

    f32 = mybir.dt.float32
    f32r = mybir.dt.float32r
    bf16 = mybir.dt.bfloat16
    fp8 = mybir.dt.float8e4
    AF = mybir.ActivationFunctionType
    OP = mybir.AluOpType
    DR = mybir.MatmulPerfMode.DoubleRow

    nc = bacc.Bacc("TRN2", target_bir_lowering=False, debug=False)

    xT = nc.dram_tensor("xT", [RPC, C, T], f32r, kind="ExternalInput")
    xTb = nc.dram_tensor("xTb", [RPC, C, T], f32, kind="ExternalInput")
    kcd = nc.dram_tensor("kcd", [RPC, HPAIR, 128, NP], bf16, kind="ExternalInput")
    vcd = nc.dram_tensor("vcd", [RPC, H, 128, KTC, HD + 1], bf16, kind="ExternalInput")
    wqkv = nc.dram_tensor("wqkv", [128, CT, 3 * C], fp8, kind="ExternalInput")
    wproj = nc.dram_tensor("wproj", [128, CT, C], fp8, kind="ExternalInput")
    wfc1 = nc.dram_tensor("wfc1", [NHT, 128, CT, 128], fp8, kind="ExternalInput")
    wfc2 = nc.dram_tensor("wfc2", [CT, 128, NHT, 128], fp8, kind="ExternalInput")
    bqkv_qk = nc.dram_tensor("bqkv_qk", [128, 16], f32, kind="ExternalInput")
    vb = nc.dram_tensor("vb", [C], f32, kind="ExternalInput")
    bfc1 = nc.dram_tensor("bfc1", [128, NHT], f32, kind="ExternalInput")
    bfc2 = nc.dram_tensor("bfc2", [128, CT], f32, kind="ExternalInput")
    # LN gains/biases, reshaped [CT, 128] host-side
    n1g = nc.dram_tensor("n1g", [CT, 128], f32r, kind="ExternalInput")
    n1b = nc.dram_tensor("n1b", [CT, 128], f32r, kind="ExternalInput")
    n2g = nc.dram_tensor("n2g", [CT, 128], f32r, kind="ExternalInput")
    n2b = nc.dram_tensor("n2b", [CT, 128], f32r, kind="ExternalInput")
    ones = nc.dram_tensor("ones", [128, 512], f32r, kind="ExternalInput")
    c32d = nc.dram_tensor("c32d", [1, HD], f32r, kind="ExternalInput")
    outT = nc.dram_tensor("outT", [RPC, C, T], f32, kind="ExternalOutput")

    from contextlib import ExitStack
    with nc.allow_low_precision(reason="deliberate fp8/bf16 staging; accumulation stays fp32 in PSUM"), \
         tile.TileContext(nc, pool_alloc_mode="queue") as tc, ExitStack() as es:
        # ---------- constants resident for the whole kernel ----------
        consts = es.enter_context(tc.tile_pool(name="consts", bufs=1))
        ones_sb = consts.tile([128, 512], f32r)
        nc.sync.dma_start(ones_sb[:], ones.ap())
        c32_sb = consts.tile([1, HD], f32r)
        nc.sync.dma_start(c32_sb[:], c32d.ap())
        gb1 = consts.tile([2, CT, 128], f32r)
        nc.sync.dma_start(gb1[0:1], n1g.ap()[None])
        nc.sync.dma_start(gb1[1:2], n1b.ap()[None])
        gb2 = consts.tile([2, CT, 128], f32r)
        nc.sync.dma_start(gb2[0:1], n2g.ap()[None])
        nc.sync.dma_start(gb2[1:2], n2b.ap()[None])
        bqkv_sb = consts.tile([128, 16], f32)  # q,k bias columns per fchunk
        nc.sync.dma_start(bqkv_sb[:], bqkv_qk.ap())
        vb_sb = consts.tile([128, 2, 512], f32)  # v bias broadcast over tokens
        for ch in range(2):
            nc.sync.dma_start(
                vb_sb[:, ch, :],
                vb.ap()[ch * 512:(ch + 1) * 512][None].to_broadcast((128, 512)))
        bfc1_sb = consts.tile([128, NHT], f32)
        nc.sync.dma_start(bfc1_sb[:], bfc1.ap())
        bfc2_sb = consts.tile([128, CT], f32)
        nc.sync.dma_start(bfc2_sb[:], bfc2.ap())
        eps_sb = consts.tile([1, 1], f32)
        nc.vector.memset(eps_sb[:], EPS)

        # resident fp8 weights (per-partition: 24 + 8 + 32 KB)
        wq_sb = consts.tile([128, CT, 3 * C], fp8)
        nc.sync.dma_start(wq_sb[:], wqkv.ap())
        wp_sb = consts.tile([128, CT, C], fp8)
        nc.sync.dma_start(wp_sb[:], wproj.ap())

        def open_pool(nm):
            cm = tc.tile_pool(name=nm, bufs=1)
            return cm, cm.__enter__()

        def close_pool(cm):
            cm.__exit__(None, None, None)

        dram_pool = es.enter_context(tc.tile_pool(name="x2d", bufs=1, space="DRAM"))
        x2ds = [[dram_pool.tile([128, T], f32, tag=f"x2d{r}_{c}", name=f"x2d{r}_{c}")
                 for c in range(CT)] for r in range(RPC)]

        # long-lived activation pools, opened in nesting order (closed LIFO):
        # g (FC2) > h2 (FC1) > oT (proj) > vn,qk (attention) > h (QKV)
        cm_g, p_g = open_pool("p_g")
        g_bufs = [p_g.tile([128, NHT, T], fp8, tag=f"g{r}", name=f"g{r}")
                  for r in range(RPC)]
        cm_h2, p_h2 = open_pool("p_h2")
        h2hi = [p_h2.tile([128, CT, T], fp8, tag=f"hh{r}", name=f"hh{r}")
                for r in range(RPC)]
        h2lo = [p_h2.tile([128, CT, T], fp8, tag=f"hl{r}", name=f"hl{r}")
                for r in range(RPC)]
        cm_oT, p_oT = open_pool("p_oT")
        oT_bufs = [p_oT.tile([128, CT, T], fp8, tag=f"oT{r}", name=f"oT{r}")
                   for r in range(RPC)]
        cm_vn, p_vn = open_pool("p_vn")
        vns = [p_vn.tile([128, KTN, H, HD + 1], bf16, tag=f"vn{r}", name=f"vn{r}")
               for r in range(RPC)]
        cm_qk, p_qk = open_pool("p_qk")
        qTs = [[p_qk.tile([128, T], bf16, tag=f"qT{r}_{c}", name=f"qT{r}_{c}")
                for c in range(CT)] for r in range(RPC)]
        kTs = [[p_qk.tile([128, T], bf16, tag=f"kT{r}_{c}", name=f"kT{r}_{c}")
                for c in range(CT)] for r in range(RPC)]
        cm_h, p_h = open_pool("p_h")
        h_bufs = [p_h.tile([128, CT, T], fp8, tag=f"h{r}", name=f"h{r}")
                  for r in range(RPC)]
        cm_xa, p_xa = open_pool("p_xa")

        def layernorm(src_tiles, dst_ap, gb, lnp, lnps, sbufs=2, dst_f32=None):
            """Feature-major layernorm; src_tiles: CT [128,T] f32r tiles;
            dst_ap(ct) returns the [128,T] output AP for tile ct."""
            s_ps = lnps.tile([1, T], f32, tag="s_ps", name="s_ps", bufs=sbufs)
            ss_ps = lnps.tile([1, T], f32, tag="ss_ps", name="ss_ps", bufs=sbufs)
            for ct in range(CT):
                nc.tensor.matmul(s_ps[:], ones_sb[:, 0:1], src_tiles[ct][:],
                                 start=(ct == 0), stop=(ct == CT - 1))
            sqs = []
            for ct in range(CT):
                sq = lnp.tile([128, T], f32r, tag="sq", name="sq", bufs=2)
                nc.gpsimd.tensor_mul(sq[:], src_tiles[ct][:].bitcast(f32),
                                     src_tiles[ct][:].bitcast(f32))
                sqs.append(sq)
            for ct in range(CT):
                nc.tensor.matmul(ss_ps[:], ones_sb[:, 0:1], sqs[ct][:],
                                 start=(ct == 0), stop=(ct == CT - 1))
            st = lnp.tile([97, T], f32, tag="st", name="st", bufs=2)
            mean, msq, var, std = st[0:1, :], st[32:33, :], st[64:65, :], st[96:97, :]
            nc.scalar.mul(mean, s_ps[:], 1.0 / C)
            nc.vector.tensor_mul(msq, mean, mean)
            nc.vector.scalar_tensor_tensor(var, ss_ps[:], 1.0 / C, msq,
                                           OP.mult, OP.subtract)
            nc.scalar.activation(std, var, AF.Sqrt, bias=eps_sb[:])
            rstd = lnp.tile([1, T], f32r, tag="rstd", name="rstd", bufs=2)
            nc.vector.reciprocal(rstd[:], std)
            nmr = lnp.tile([2, T], f32r, tag="nmr", name="nmr", bufs=2)
            nc.vector.scalar_tensor_tensor(nmr[0:1, :], mean, -1.0,
                                           rstd[:].bitcast(f32), OP.mult, OP.mult)
            nc.sync.dma_start(nmr[1:2, :], ones.ap()[0:1, :])
            for ct in range(CT):
                a_ps = lnps.tile([128, T], f32, tag="a_ps", name="a_ps", bufs=2)
                nc.tensor.matmul(a_ps[:], gb[0:1, ct, :], rstd[:],
                                 start=True, stop=True)
                b_ps = lnps.tile([128, T], f32, tag="b_ps", name="b_ps", bufs=2)
                nc.tensor.matmul(b_ps[:], gb[:, ct, :], nmr[:],
                                 start=True, stop=True)
                t1 = lnp.tile([128, T], f32, tag="t1", name="t1", bufs=2)
                nc.vector.tensor_mul(t1[:], src_tiles[ct][:].bitcast(f32), a_ps[:])
                if dst_f32 is None:
                    nc.vector.tensor_add(dst_ap(ct), t1[:], b_ps[:])
                else:
                    nc.vector.tensor_add(dst_f32(ct), t1[:], b_ps[:])

        # ================= LN1 -> h fp8 =================
        with tc.tile_pool(name="ln1", bufs=1) as lnp, \
             tc.tile_pool(name="ln1ps", bufs=1, space="PSUM") as lnps:
            for r in range(RPC):
                xTs = [p_xa.tile([128, T], f32r, tag=f"xT{ct}",
                                 name=f"xT{r}_{ct}", bufs=1) for ct in range(CT)]
                for ct in range(CT):
                    nc.sync.dma_start(xTs[ct][:], xT.ap()[r, ct * 128:(ct + 1) * 128, :])
                layernorm(xTs, lambda ct, r=r: h_bufs[r][:, ct, :], gb1, lnp, lnps)
        close_pool(cm_xa)

        # ================= QKV (fp8 DoubleRow) =================
        with tc.tile_pool(name="qkv_ps", bufs=1, space="PSUM") as qps:
            for r in range(RPC):
                nc.vector.memset(vns[r][:, :, :, HD:HD + 1], 1.0)
            for r in range(RPC):
                for fc in range(16):   # q chunks 0-7, k chunks 8-15
                    ps = qps.tile([128, T], f32, tag="mm", name="mm", bufs=3)
                    for c in range(CT // 2):
                        nc.tensor.matmul(
                            ps[:], wq_sb[:, c::4, fc * 128:(fc + 1) * 128],
                            h_bufs[r][:, c::4, :], start=(c == 0),
                            stop=(c == CT // 2 - 1), perf_mode=DR)
                    dst = qTs[r][fc] if fc < 8 else kTs[r][fc - 8]
                    nc.vector.tensor_scalar(
                        dst[:], ps[:], 1.0 / SW, bqkv_sb[:, fc:fc + 1],
                        OP.mult, OP.add)
                # v: token-major, h as stationary
                for ch in range(4):
                    for tt in range(KTN):
                        ps = qps.tile([128, 256], f32, tag="mmv", name="mmv", bufs=3)
                        for c in range(CT // 2):
                            nc.tensor.matmul(
                                ps[:], h_bufs[r][:, c::4, tt * 128:(tt + 1) * 128],
                                wq_sb[:, c::4, 2048 + ch * 256:2048 + (ch + 1) * 256],
                                start=(c == 0), stop=(c == CT // 2 - 1), perf_mode=DR)
                        nc.vector.scalar_tensor_tensor(
                            vns[r][:, tt, ch * 4:(ch + 1) * 4, 0:HD],
                            ps[:].rearrange("p (h d) -> p h d", h=4), 1.0 / SW,
                            vb_sb[:, ch // 2, ch % 2 * 256:(ch % 2 + 1) * 256]
                            .rearrange("p (h d) -> p h d", h=4),
                            OP.mult, OP.add)
        close_pool(cm_h)

        # ================= Attention =================
        with tc.tile_pool(name="attn_kv", bufs=1) as akv, \
             tc.tile_pool(name="attn_sb", bufs=1) as asb, \
             tc.tile_pool(name="attn_ps", bufs=1, space="PSUM") as mps:
            for hp in range(HPAIR):
                for r in range(RPC):
                    kc_t = akv.tile([128, NP], bf16, tag="kc", name="kc", bufs=3)
                    nc.sync.dma_start(kc_t[:], kcd.ap()[r, hp])
                    vcs = [akv.tile([128, KTC, HD + 1], bf16, tag="vcc",
                                    name="vcc", bufs=6) for _ in range(2)]
                    for hh in range(2):
                        nc.sync.dma_start(vcs[hh][:], vcd.ap()[r, 2 * hp + hh])
                    pv = [mps.tile([HD + 1, T], f32, tag=f"pv{hh}",
                                   name=f"pv{hh}", bufs=1) for hh in range(2)]
                    for kt in range(KTA):
                        if kt < KTC:
                            lA = kc_t[0:64, kt * 128:(kt + 1) * 128]
                            lB = kc_t[64:128, kt * 128:(kt + 1) * 128]
                        else:
                            ktn = kt - KTC
                            lA = kTs[r][hp][0:64, ktn * 128:(ktn + 1) * 128]
                            lB = kTs[r][hp][64:128, ktn * 128:(ktn + 1) * 128]
                        s = mps.tile([128, 2, T], f32, tag="s", name="s", bufs=2)
                        nc.tensor.matmul(s[:, 0, :], lA, qTs[r][hp][0:64, :],
                                         start=True, stop=True, tile_position=(0, 0))
                        nc.tensor.matmul(s[:, 1, :], lB, qTs[r][hp][64:128, :],
                                         start=True, stop=True, tile_position=(64, 0))
                        p_t = asb.tile([128, 2, T], bf16, tag="p", name="p", bufs=4)
                        nc.scalar.activation(p_t[:], s[:], AF.Exp, scale=SCALE)
                        for hh in range(2):
                            lv = vcs[hh][:, kt, :] if kt < KTC else \
                                vns[r][:, kt - KTC, 2 * hp + hh, :]
                            nc.tensor.matmul(pv[hh][:], lv, p_t[:, hh, :],
                                             start=(kt == 0), stop=(kt == KTA - 1))
                    for hh in range(2):
                        rd = asb.tile([1, T], f32r, tag="rd", name="rd", bufs=2)
                        nc.vector.reciprocal(rd[:], pv[hh][HD:HD + 1, :])
                        bc = mps.tile([HD, T], f32, tag="bc", name="bc", bufs=1)
                        nc.tensor.matmul(bc[:], c32_sb[:], rd[:],
                                         start=True, stop=True)
                        bc_sb = asb.tile([HD, T], f32, tag="bcs", name="bcs", bufs=2)
                        nc.vector.tensor_copy(bc_sb[:], bc[:])
                        nc.vector.tensor_mul(
                            oT_bufs[r][64 * hh:64 * (hh + 1), hp, :],
                            pv[hh][0:HD, :], bc_sb[:])
        close_pool(cm_qk)
        close_pool(cm_vn)

        # ================= Proj (fp8 DR) + residual + LN2 + hi/lo =================
        with tc.tile_pool(name="p_x2", bufs=1) as p_x2, \
             tc.tile_pool(name="xbp", bufs=1) as xbp_pool, \
             tc.tile_pool(name="ln2", bufs=1) as lnp2, \
             tc.tile_pool(name="proj_ps", bufs=1, space="PSUM") as pps:
            for r in range(RPC):
                x2Ts = [p_x2.tile([128, T], f32r, tag=f"x2T{ct}",
                                  name=f"x2T{r}_{ct}", bufs=1) for ct in range(CT)]
                h2f = [p_x2.tile([128, T], f32r, tag=f"h2f{ct}",
                                 name=f"h2f{r}_{ct}", bufs=1) for ct in range(CT)]
                for co in range(CT):
                    ps = pps.tile([128, T], f32, tag="proj", name="proj", bufs=2)
                    for c in range(CT // 2):
                        nc.tensor.matmul(
                            ps[:], wp_sb[:, c::4, co * 128:(co + 1) * 128],
                            oT_bufs[r][:, c::4, :], start=(c == 0),
                            stop=(c == CT // 2 - 1), perf_mode=DR)
                    xb = xbp_pool.tile([128, T], f32, tag="xb", name="xb", bufs=4)
                    nc.sync.dma_start(
                        xb[:], xTb.ap()[r, co * 128:(co + 1) * 128, :])
                    nc.vector.scalar_tensor_tensor(
                        x2Ts[co][:], ps[:], 1.0 / (SO * SW),
                        xb[:], OP.mult, OP.add)
                    nc.gpsimd.dma_start(x2ds[r][co][:], x2Ts[co][:].bitcast(f32))
                layernorm(x2Ts, None, gb2, lnp2, pps, sbufs=1,
                          dst_f32=lambda ct, h2f=h2f: h2f[ct][:])
                for ct in range(CT):
                    nc.vector.tensor_scalar(
                        h2hi[r][:, ct, :], h2f[ct][:].bitcast(f32),
                        SH2, None, OP.mult)
                    nc.vector.scalar_tensor_tensor(
                        h2lo[r][:, ct, :], h2f[ct][:].bitcast(f32), SH2,
                        h2hi[r][:, ct, :], OP.mult, OP.subtract)
        close_pool(cm_oT)

        # ================= FC1 (fp8 DR, hi+lo) =================
        with tc.tile_pool(name="w1s", bufs=1) as w1_pool, \
             tc.tile_pool(name="fc1_ps", bufs=1, space="PSUM") as f1ps:
            for ht in range(NHT):
                w1t = w1_pool.tile([128, CT, 128], fp8, tag="w1", name="w1", bufs=4)
                nc.sync.dma_start(w1t[:], wfc1.ap()[ht])
                for r in range(RPC):
                    ps = f1ps.tile([128, T], f32, tag="fc1", name="fc1", bufs=4)
                    for c in range(CT // 2):
                        nc.tensor.matmul(
                            ps[:], w1t[:, c::4, :],
                            h2hi[r][:, c::4, :], start=(c == 0), stop=False,
                            perf_mode=DR)
                    for c in range(CT // 2):
                        nc.tensor.matmul(
                            ps[:], w1t[:, c::4, :],
                            h2lo[r][:, c::4, :], start=False,
                            stop=(c == CT // 2 - 1), perf_mode=DR)
                    nc.scalar.activation(g_bufs[r][:, ht, :], ps[:], AF.Gelu,
                                         bias=bfc1_sb[:, ht:ht + 1],
                                         scale=1.0 / (SW * SH2))
        close_pool(cm_h2)

        # ================= FC2 (fp8 DR) + residual -> out =================
        with tc.tile_pool(name="w2", bufs=1) as w2_pool, \
             tc.tile_pool(name="xb2", bufs=1) as xb2_pool, \
             tc.tile_pool(name="osb", bufs=1) as osb, \
             tc.tile_pool(name="fc2_ps", bufs=1, space="PSUM") as f2ps:
            for co in range(CT):
                w2t = w2_pool.tile([128, NHT, 128], fp8, tag="w2", name="w2", bufs=3)
                nc.sync.dma_start(w2t[:], wfc2.ap()[co])
                for r in range(RPC):
                    ps = f2ps.tile([128, T], f32, tag="fc2", name="fc2", bufs=3)
                    for c in range(NHT // 2):
                        nc.tensor.matmul(
                            ps[:], w2t[:, c::16, :], g_bufs[r][:, c::16, :],
                            start=(c == 0), stop=(c == NHT // 2 - 1), perf_mode=DR)
                    xb2 = xb2_pool.tile([128, T], f32, tag="xb2", name="xb2", bufs=3)
                    nc.gpsimd.dma_start(xb2[:], x2ds[r][co][:])
                    tmp = osb.tile([128, T], f32, tag="tmp", name="tmp", bufs=2)
                    nc.vector.tensor_scalar(
                        tmp[:], ps[:], 1.0 / SW, bfc2_sb[:, co:co + 1],
                        OP.mult, OP.add)
                    ot = osb.tile([128, T], f32, tag="ot", name="ot", bufs=2)
                    nc.gpsimd.tensor_add(ot[:], tmp[:], xb2[:])
                    nc.sync.dma_start(outT.ap()[r, co * 128:(co + 1) * 128, :], ot[:])
        close_pool(cm_g)

    nc.compile()
    return nc


class _Runner:
    """Hold the compiled PJRT executable (mirrors bass2jax.run_bass_via_pjrt)."""

    def __init__(self, nc, n_cores):
        import jax
        from jax.sharding import Mesh, PartitionSpec
        from jax.experimental.shard_map import shard_map
        import concourse.mybir as mybir
        from concourse.bass2jax import (
            install_neuronx_cc_hook, partition_id_tensor, _bass_exec_p)

        install_neuronx_cc_hook()
        self.jax = jax
        self.n_cores = n_cores
        partition_name = nc.partition_id_tensor.name if nc.partition_id_tensor else None
        in_names, out_names, out_avals, zero_outs = [], [], [], []
        for alloc in nc.m.functions[0].allocations:
            if not isinstance(alloc, mybir.MemoryLocationSet):
                continue
            name = alloc.memorylocations[0].name
            if alloc.kind == "ExternalInput":
                if name != partition_name:
                    in_names.append(name)
            elif alloc.kind == "ExternalOutput":
                shape = tuple(alloc.tensor_shape)
                dtype = mybir.dt.np(alloc.dtype)
                out_names.append(name)
                out_avals.append(jax.core.ShapedArray(shape, dtype))
                zero_outs.append(np.zeros(shape, dtype))
        self.in_names, self.out_names = in_names, out_names
        self.out_avals, self.zero_outs = out_avals, zero_outs
        self.n_params = len(in_names)
        all_names = in_names + out_names
        if partition_name is not None:
            all_names.append(partition_name)

        def _body(*args):
            operands = list(args)
            if partition_name is not None:
                operands.append(partition_id_tensor())
            return tuple(
                _bass_exec_p.bind(
                    *operands,
                    out_avals=tuple(out_avals),
                    in_names=tuple(all_names),
                    out_names=tuple(out_names),
                    lowering_input_output_aliases=(),
                    sim_require_finite=True,
                    sim_require_nnan=True,
                    nc=nc,
                ))

        devices = jax.devices()[:n_cores]
        assert len(devices) == n_cores, f"need {n_cores} cores, have {len(jax.devices())}"
        mesh = Mesh(np.asarray(devices), ("core",))
        n_outs = len(out_names)
        self._fn = jax.jit(
            shard_map(_body, mesh=mesh,
                      in_specs=(PartitionSpec("core"),) * (self.n_params + n_outs),
                      out_specs=(PartitionSpec("core"),) * n_outs,
                      check_rep=False),
            keep_unused=True)

    def prepare(self, in_maps):
        np_ = np
        per_core = [[np_.asarray(m[n]) for n in self.in_names] for m in in_maps]
        concat_in = [
            np_.concatenate([per_core[c][i] for c in range(self.n_cores)], axis=0)
            for i in range(self.n_params)]
        concat_zeros = [
            np_.zeros((self.n_cores * z.shape[0], *z.shape[1:]), z.dtype)
            for z in self.zero_outs]
        return self.jax.device_put(concat_in + concat_zeros)

    def run(self, prepared):
        out = self._fn(*prepared)
        self.jax.block_until_ready(out)
        return out

    def results(self, out_arrs):
        return [
            {name: np.asarray(out_arrs[i]).reshape(
                self.n_cores, *self.out_avals[i].shape)[c]
             for i, name in enumerate(self.out_names)}
            for c in range(self.n_cores)]


def _get_runner():
    if "runner" not in _state:
        nc = _build_module()
        _state["nc"] = nc
        _state["runner"] = _Runner(nc, NCORES)
    return _state["runner"]


def _prepare_in_maps(x, cache_k, cache_v, update_mask, qkv_w, qkv_b, proj_w,
                     proj_b, n1_g, n1_b, n2_g, n2_b, fc1_w, fc1_b, fc2_w, fc2_b):
    bf = ml_dtypes.bfloat16
    fp8 = ml_dtypes.float8_e4m3
    f32 = np.float32

    x = np.asarray(x, f32)
    xT = np.ascontiguousarray(np.swapaxes(x, 1, 2))                  # [B,C,T]
    proj_b = np.asarray(proj_b, f32)
    xTb = np.ascontiguousarray(xT + proj_b[None, :, None])

    # gather the kept (unmasked) cache slots: exactly NP kept per row
    um = np.asarray(update_mask, bool)
    keep = np.stack([np.nonzero(~um[b])[0] for b in range(B)])       # [B, NP]
    assert keep.shape == (B, NP)
    bidx = np.arange(B)[:, None]
    ck = np.asarray(cache_k, f32)[bidx[:, None], np.arange(H)[None, :, None],
                                  keep[:, None, :]]                  # [B,H,NP,HD]
    cv = np.asarray(cache_v, f32)[bidx[:, None], np.arange(H)[None, :, None],
                                  keep[:, None, :]]                  # [B,H,NP,HD]
    # kc: [B, HPAIR, 128(hh,d), NP] bf16
    kcd = np.ascontiguousarray(
        ck.reshape(B, HPAIR, 2, NP, HD).transpose(0, 1, 2, 4, 3)
        .reshape(B, HPAIR, 128, NP)).astype(bf)
    # vc: [B, H, 128, KTC, HD+1] bf16 (p-major so the DMA is contiguous)
    cv1 = np.concatenate([cv, np.ones((B, H, NP, 1), f32)], axis=3)  # [B,H,NP,65]
    vcd = np.ascontiguousarray(
        cv1.reshape(B, H, KTC, 128, HD + 1).transpose(0, 1, 3, 2, 4)).astype(bf)

    def pack_w(w, scale):
        # [K, F] -> [128, K//128, F] fp8, pre-scaled
        K, F = w.shape
        return np.ascontiguousarray(
            (np.asarray(w, f32) * scale).reshape(K // 128, 128, F)
            .transpose(1, 0, 2)).astype(fp8)

    qkv_b = np.asarray(qkv_b, f32)
    shared = dict(
        wqkv=pack_w(np.asarray(qkv_w, f32), SW),
        wproj=pack_w(np.asarray(proj_w, f32), SW),
        wfc1=np.ascontiguousarray(
            pack_w(np.asarray(fc1_w, f32), SW)
            .reshape(128, CT, NHT, 128).transpose(2, 0, 1, 3)),     # [NHT,128,CT,128]
        wfc2=np.ascontiguousarray(
            pack_w(np.asarray(fc2_w, f32), SW)
            .reshape(128, NHT, CT, 128).transpose(2, 0, 1, 3)),      # [CT,128,NHT,128]
        bqkv_qk=np.ascontiguousarray(
            qkv_b[0:2048].reshape(16, 128).T),                       # [128,16]
        vb=qkv_b[2048:3072].copy(),
        bfc1=np.ascontiguousarray(
            np.asarray(fc1_b, f32).reshape(NHT, 128).T),             # [128,NHT]
        bfc2=np.ascontiguousarray(
            np.asarray(fc2_b, f32).reshape(CT, 128).T),              # [128,CT]
        n1g=np.asarray(n1_g, f32).reshape(CT, 128),
        n1b=np.asarray(n1_b, f32).reshape(CT, 128),
        n2g=np.asarray(n2_g, f32).reshape(CT, 128),
        n2b=np.asarray(n2_b, f32).reshape(CT, 128),
        ones=np.ones((128, 512), f32),
        c32d=np.full((1, HD), SO, f32),
    )
    in_maps = []
    for c in range(NCORES):
        s = slice(c * RPC, (c + 1) * RPC)
        in_maps.append(dict(shared, xT=xT[s], xTb=xTb[s], kcd=kcd[s], vcd=vcd[s]))
    return in_maps


def kernel(**inputs) -> np.ndarray:
    runner = _get_runner()
    in_maps = _prepare_in_maps(**inputs)
    prepared = runner.prepare(in_maps)
    out = runner.run(prepared)
    res = runner.results(out)
    full = np.empty((B, NP, C), np.float32)
    for c in range(NCORES):
        for r in range(RPC):
            full[c * RPC + r] = res[c]["outT"][r].T
    return full
